# revision 19
# baseline (speedup 1.0000x reference)
"""ViT-style attention with decomposed relative position embeddings on 8 TRN2
NeuronCores. Data-parallel over batch (B=8 -> 1 image per core); weights and
the small rel-pos tables are replicated.

Per-core computation (one image, T=1024 tokens, C=768, 12 heads x 64):
  - qk^T GEMM:  qk^T[o, t] = w_qk^T . x^T   (q pre-scaled by 1/8 host-side)
  - v GEMM:     V[t, o] = x . w_v + b_v
  - rel-pos fold: logits = [q; rel_h_pre; rel_w_pre] . [k; onehot_h; onehot_w]
    so the decomposed rel-pos additions ride in the same 128-deep matmul
    contraction as q.k (the head-dim is only 64, so the extra 64 contraction
    rows are free on the 128x128 PE array).
  - S^T = Kext^T . Qext per head, exp on ScalarE -> P^T (bf16)
  - out^T[n, q] = Vaug^T . P^T accumulated over k-chunks; head h carries a
    one-hot ones-column at position 64+h, so the softmax denominator lands at
    psum row 64+h and all 12 denominators can be collected shift-free into
    one [76, T] tensor for a single batched reciprocal.
  - normalize via gpsimd partition-broadcast + DVE multiply
  - proj GEMM + bias -> out

All matmuls run in bf16 (fp32 PSUM accumulation).
"""

import numpy as np
import ml_dtypes

BF16 = ml_dtypes.bfloat16

B, H, W, C = 8, 32, 32, 768
NH, HD, T = 12, 64, 1024
N_CORES = 8

_cache = {}


def _bf(a):
    return np.ascontiguousarray(np.asarray(a, dtype=np.float32)).astype(BF16)


def _f32(a):
    return np.ascontiguousarray(np.asarray(a, dtype=np.float32))


def _build_nc():
    if "nc" in _cache:
        return _cache["nc"]

    import concourse.mybir as mybir
    import concourse.tile as tile
    from concourse import bacc

    f32 = mybir.dt.float32
    bf16 = mybir.dt.bfloat16
    EXP = mybir.ActivationFunctionType.Exp

    nc = bacc.Bacc("TRN2", target_bir_lowering=False, debug=False)

    # ---- DRAM I/O ----
    xT_d = nc.dram_tensor("xT", [C, T], bf16, kind="ExternalInput")
    wqk_d = nc.dram_tensor("w_qk", [C, 2 * C], bf16, kind="ExternalInput")
    wv_d = nc.dram_tensor("w_v", [C, C], bf16, kind="ExternalInput")
    wp_d = nc.dram_tensor("w_p", [C, C], bf16, kind="ExternalInput")
    bqk_d = nc.dram_tensor("b_qk", [128, 12], f32, kind="ExternalInput")
    bv_d = nc.dram_tensor("b_v", [1, C], f32, kind="ExternalInput")
    bp_d = nc.dram_tensor("b_p", [1, C], f32, kind="ExternalInput")
    relt_d = nc.dram_tensor("relt", [64, 2048], bf16, kind="ExternalInput")
    oneh_d = nc.dram_tensor("onehot", [64, T], bf16, kind="ExternalInput")
    out_d = nc.dram_tensor("out", [T, C], f32, kind="ExternalOutput")

    with tile.TileContext(nc) as tc:
        with tc.tile_pool(name="const", bufs=1) as cp:
            # ---- persistent SBUF tensors ----
            xT = cp.tile([128, 6, T], bf16, tag="xT")
            wqk = cp.tile([128, 6, 2 * C], bf16, tag="wqk")
            wv = cp.tile([128, 6, C], bf16, tag="wv")
            wpr = cp.tile([128, 6, C], bf16, tag="wpr")
            bqk = cp.tile([128, 12], f32, tag="bqk")
            bv_row = cp.tile([1, C], f32, tag="bv_row")
            bp_row = cp.tile([1, C], f32, tag="bp_row")
            bv_bc = cp.tile([128, C], f32, tag="bv_bc")
            bp_bc = cp.tile([128, C], f32, tag="bp_bc")
            relt = cp.tile([64, 2048], bf16, tag="relt")
            qext = cp.tile([128, NH, 32, 32], bf16, tag="qext")
            kext = cp.tile([128, NH, T], bf16, tag="kext")
            vaug = cp.tile([128, 8, NH, 65], bf16, tag="vaug")
            yall = cp.tile([128, 6, T], bf16, tag="yall")

            # ---- input DMAs ----
            for c in range(6):
                nc.sync.dma_start(xT[:, c, :], xT_d[c * 128:(c + 1) * 128, :])
            for c in range(6):
                nc.sync.dma_start(wqk[:, c, :], wqk_d[c * 128:(c + 1) * 128, :])
            for c in range(6):
                nc.sync.dma_start(wv[:, c, :], wv_d[c * 128:(c + 1) * 128, :])
            for c in range(6):
                nc.sync.dma_start(wpr[:, c, :], wp_d[c * 128:(c + 1) * 128, :])
            nc.sync.dma_start(bqk[:], bqk_d[:])
            nc.sync.dma_start(bv_row[:], bv_d[:])
            nc.sync.dma_start(bp_row[:], bp_d[:])
            nc.sync.dma_start(relt[:], relt_d[:])
            nc.gpsimd.partition_broadcast(bv_bc[:], bv_row[:])
            nc.gpsimd.partition_broadcast(bp_bc[:], bp_row[:])
            # onehot block straight into each head's Kext rows 64:128
            for h in range(NH):
                nc.sync.dma_start(kext[64:128, h, :], oneh_d[:])
            # ones column of Vaug (softmax denominator)
            nc.gpsimd.memset(vaug[:, :, :, 64:65], 1.0)

            # ======== phase 1a: q projections (o-tiles 0-5) ========
            def qk_otile(ps, ot):
                acc = ps.tile([128, T], f32, tag="qk")
                for c in range(6):
                    for hf in range(2):
                        nc.tensor.matmul(
                            acc[:, hf * 512:(hf + 1) * 512],
                            wqk[:, c, ot * 128:(ot + 1) * 128],
                            xT[:, c, hf * 512:(hf + 1) * 512],
                            start=(c == 0), stop=(c == 5),
                        )
                is_q = ot < 6
                hp = ot if is_q else ot - 6  # head pair index
                for half in range(2):
                    head = 2 * hp + half
                    src = acc[64 * half:64 * (half + 1), :]
                    bias = bqk[64 * half:64 * (half + 1), ot:ot + 1]
                    if is_q:
                        dst = qext[0:64, head, :, :]
                    else:
                        dst = kext[0:64, head, :]
                    nc.vector.tensor_scalar_add(dst, src, bias)

            with tc.tile_pool(name="ps_qk", bufs=2, space="PSUM") as ps_qk:
                for ot in range(6):
                    qk_otile(ps_qk, ot)

                # ==== phase 1b: rel-pos tables, overlapped with k o-tiles ====
                # Bands m=2 (rel_h -> qext rows 64:96) and m=3 (rel_w ->
                # rows 96:128) so every evacuation is partition-aligned and
                # can run on either ScalarE or VectorE. k o-tiles are
                # interleaved so the PE stays warm while DVE/ACT drain the
                # rel psum tiles.
                k_sched = {1: 6, 4: 7, 7: 8, 10: 9, 12: 10, 14: 11}
                with tc.tile_pool(name="ps_rel", bufs=2, space="PSUM") as ps_rel:
                    for i in range(16):
                        if i in k_sched:
                            qk_otile(ps_qk, k_sched[i])
                        accr = ps_rel.tile([128, 2, 512], f32, tag="rel")
                        for g in range(2):
                            qx = 2 * i + g
                            for tbl in range(2):
                                m = 2 + tbl
                                lhsT = relt[0:64, tbl * 1024 + qx * 32:
                                            tbl * 1024 + qx * 32 + 32]
                                rhs = (qext[0:64, :, qx, :] if tbl == 0
                                       else qext[0:64, :, :, qx])
                                nc.tensor.matmul(
                                    accr[32 * m:32 * (m + 1), g, 0:NH * 32],
                                    lhsT, rhs,
                                    start=True, stop=True,
                                    tile_position=(0, 32 * m),
                                )
                        q0 = 2 * i
                        src_h = accr[64:96, :, 0:NH * 32]
                        dst_h = qext[64:96, :, q0:q0 + 2, :].rearrange(
                            "p h a b -> p a h b")
                        src_w = accr[96:128, :, 0:NH * 32]
                        dst_w = qext[96:128, :, :, q0:q0 + 2].rearrange(
                            "p h a b -> p b h a")
                        if i % 2 == 0:
                            nc.scalar.copy(dst_h, src_h)
                            nc.vector.tensor_copy(dst_w, src_w)
                        else:
                            nc.vector.tensor_copy(dst_h, src_h)
                            nc.scalar.copy(dst_w, src_w)

            # ======== phase 1c: v GEMM ========
            with tc.tile_pool(name="ps_v", bufs=2, space="PSUM") as ps_v:
                for tt in range(8):
                    accv = ps_v.tile([128, C], f32, tag="v")
                    for c in range(6):
                        nc.tensor.matmul(
                            accv[:, 0:512],
                            xT[:, c, tt * 128:(tt + 1) * 128],
                            wv[:, c, 0:512],
                            start=(c == 0), stop=(c == 5),
                        )
                        nc.tensor.matmul(
                            accv[:, 512:768],
                            xT[:, c, tt * 128:(tt + 1) * 128],
                            wv[:, c, 512:768],
                            start=(c == 0), stop=(c == 5),
                        )
                    nc.vector.tensor_add(
                        vaug[:, tt, :, 0:64], accv[:], bv_bc[:])

            # ================= phase 2b: attention per head =================
            with tc.tile_pool(name="ps_s", bufs=3, space="PSUM") as ps_s, \
                 tc.tile_pool(name="ps_pv", bufs=1, space="PSUM") as ps_pv, \
                 tc.tile_pool(name="attn_sb", bufs=2) as asb:
                for h in range(NH):
                    p_t = asb.tile([128, 8, T], bf16, tag="P")
                    for kt in range(8):
                        accs = ps_s.tile([128, T], f32, tag="S")
                        for hf in range(2):
                            nc.tensor.matmul(
                                accs[:, hf * 512:(hf + 1) * 512],
                                kext[:, h, kt * 128:(kt + 1) * 128],
                                qext[:, h, hf * 16:(hf + 1) * 16, :],
                                start=True, stop=True,
                            )
                        nc.scalar.activation(p_t[:, kt, :], accs[:], EXP)
                    accp = ps_pv.tile([65, T], f32, tag="PV")
                    for kt in range(8):
                        for hf in range(2):
                            nc.tensor.matmul(
                                accp[:, hf * 512:(hf + 1) * 512],
                                vaug[:, kt, h, :],
                                p_t[:, kt, hf * 512:(hf + 1) * 512],
                                start=(kt == 0), stop=(kt == 7),
                            )
                    # normalization: reciprocal costs ~6.4 DVE cycles/elem,
                    # so spread the 1024 denominators over 32 lanes via a
                    # small DMA reshape (32 descriptors each way) first.
                    d_row = asb.tile([1, T], f32, tag="d")
                    d_sq = asb.tile([32, 32], f32, tag="dsq")
                    r_row = asb.tile([1, T], f32, tag="r")
                    r_bc = asb.tile([64, T], f32, tag="rbc")
                    nc.vector.tensor_copy(d_row[:], accp[64:65, :])
                    nc.sync.dma_start(d_sq[:], d_row[:])
                    nc.vector.reciprocal(d_sq[:], d_sq[:])
                    nc.sync.dma_start(r_row[:], d_sq[:])
                    nc.gpsimd.partition_broadcast(r_bc[:], r_row[:])
                    nc.vector.tensor_mul(
                        yall[64 * (h % 2):64 * (h % 2 + 1), h // 2, :],
                        accp[0:64, :], r_bc[:])

            # ================= phase 3: output projection =================
            with tc.tile_pool(name="ps_pr", bufs=2, space="PSUM") as ps_pr, \
                 tc.tile_pool(name="proj_sb", bufs=2) as psb:
                for tt in range(8):
                    accz = ps_pr.tile([128, C], f32, tag="Z")
                    for p in range(6):
                        nc.tensor.matmul(
                            accz[:, 0:512],
                            yall[:, p, tt * 128:(tt + 1) * 128],
                            wpr[:, p, 0:512],
                            start=(p == 0), stop=(p == 5),
                        )
                        nc.tensor.matmul(
                            accz[:, 512:768],
                            yall[:, p, tt * 128:(tt + 1) * 128],
                            wpr[:, p, 512:768],
                            start=(p == 0), stop=(p == 5),
                        )
                    z_t = psb.tile([128, C], f32, tag="Zt")
                    nc.vector.tensor_add(z_t[:], accz[:], bp_bc[:])
                    nc.sync.dma_start(out_d[tt * 128:(tt + 1) * 128, :], z_t[:])

    nc.compile()
    _cache["nc"] = nc
    return nc


def _host_prep(x, w_qkv, b_qkv, w_proj, b_proj, rel_pos_h, rel_pos_w):
    scale = HD ** -0.5
    w_qkv = _f32(w_qkv)
    b_qkv = _f32(b_qkv)

    w_qk = w_qkv[:, : 2 * C].copy()
    w_qk[:, :C] *= scale
    b_qk_flat = b_qkv[: 2 * C].copy()
    b_qk_flat[:C] *= scale
    b_qk = np.ascontiguousarray(b_qk_flat.reshape(12, 128).T)  # [128, 12]

    # relt [64, 2048]: cols tbl*1024 + qx*32 + j -> 8*rel_pos[qx - j + 31, :]
    idx = np.arange(32)[:, None] - np.arange(32)[None, :] + 31  # [qx, j]
    relt = np.concatenate(
        [
            (8.0 * _f32(rel_pos_h))[idx].transpose(2, 0, 1).reshape(64, 1024),
            (8.0 * _f32(rel_pos_w))[idx].transpose(2, 0, 1).reshape(64, 1024),
        ],
        axis=1,
    )

    k = np.arange(T)
    onehot = np.zeros((64, T), np.float32)
    onehot[k // 32, k] = 1.0
    onehot[32 + k % 32, k] = 1.0

    shared = {
        "w_qk": _bf(w_qk),
        "w_v": _bf(w_qkv[:, 2 * C:]),
        "w_p": _bf(w_proj),
        "b_qk": _f32(b_qk),
        "b_v": _f32(b_qkv[2 * C:])[None, :],
        "b_p": _f32(b_proj)[None, :],
        "relt": _bf(relt),
        "onehot": _bf(onehot),
    }
    x = _f32(x)
    in_maps = []
    for i in range(N_CORES):
        m = dict(shared)
        m["xT"] = _bf(x[i].reshape(T, C).T)
        in_maps.append(m)
    return in_maps


def kernel(x, w_qkv, b_qkv, w_proj, b_proj, rel_pos_h, rel_pos_w):
    from concourse.bass_utils import run_bass_kernel_spmd

    nc = _build_nc()
    in_maps = _host_prep(x, w_qkv, b_qkv, w_proj, b_proj, rel_pos_h, rel_pos_w)
    res = run_bass_kernel_spmd(nc, in_maps, core_ids=list(range(N_CORES)))
    out = np.stack([_f32(res.results[i]["out"]) for i in range(N_CORES)])
    return out.reshape(B, H, W, C)


# revision 20
# speedup vs baseline: 1.2119x; 1.2119x over previous
"""ViT-style attention with decomposed relative position embeddings on 8 TRN2
NeuronCores. Data-parallel over batch (B=8 -> 1 image per core); weights and
the small rel-pos tables are replicated.

Per-core computation (one image, T=1024 tokens, C=768, 12 heads x 64):
  - qk^T GEMM:  qk^T[o, t] = w_qk^T . x^T   (q pre-scaled by 1/8 host-side)
  - v GEMM:     V[t, o] = x . w_v + b_v
  - rel-pos fold: logits = [q; rel_h_pre; rel_w_pre] . [k; onehot_h; onehot_w]
    so the decomposed rel-pos additions ride in the same 128-deep matmul
    contraction as q.k (the head-dim is only 64, so the extra 64 contraction
    rows are free on the 128x128 PE array).
  - S^T = Kext^T . Qext per head, exp on ScalarE -> P^T (bf16)
  - out^T[n, q] = Vaug^T . P^T accumulated over k-chunks; head h carries a
    one-hot ones-column at position 64+h, so the softmax denominator lands at
    psum row 64+h and all 12 denominators can be collected shift-free into
    one [76, T] tensor for a single batched reciprocal.
  - normalize via gpsimd partition-broadcast + DVE multiply
  - proj GEMM + bias -> out

All matmuls run in bf16 (fp32 PSUM accumulation).
"""

import numpy as np
import ml_dtypes

BF16 = ml_dtypes.bfloat16

B, H, W, C = 8, 32, 32, 768
NH, HD, T = 12, 64, 1024
N_CORES = 8

_cache = {}


def _bf(a):
    return np.ascontiguousarray(np.asarray(a, dtype=np.float32)).astype(BF16)


def _f32(a):
    return np.ascontiguousarray(np.asarray(a, dtype=np.float32))


def _build_nc():
    if "nc" in _cache:
        return _cache["nc"]

    import concourse.mybir as mybir
    import concourse.tile as tile
    from concourse import bacc

    f32 = mybir.dt.float32
    bf16 = mybir.dt.bfloat16
    EXP = mybir.ActivationFunctionType.Exp

    nc = bacc.Bacc("TRN2", target_bir_lowering=False, debug=False)

    # ---- DRAM I/O ----
    xT_d = nc.dram_tensor("xT", [C, T], bf16, kind="ExternalInput")
    wqk_d = nc.dram_tensor("w_qk", [C, 2 * C], bf16, kind="ExternalInput")
    wv_d = nc.dram_tensor("w_v", [C, C], bf16, kind="ExternalInput")
    wp_d = nc.dram_tensor("w_p", [C, C], bf16, kind="ExternalInput")
    bqk_d = nc.dram_tensor("b_qk", [128, 12], f32, kind="ExternalInput")
    bv_d = nc.dram_tensor("b_v", [1, C], f32, kind="ExternalInput")
    bp_d = nc.dram_tensor("b_p", [1, C], f32, kind="ExternalInput")
    relt_d = nc.dram_tensor("relt", [64, 2048], bf16, kind="ExternalInput")
    oneh_d = nc.dram_tensor("onehot", [64, T], bf16, kind="ExternalInput")
    out_d = nc.dram_tensor("out", [T, C], f32, kind="ExternalOutput")

    with tile.TileContext(nc) as tc:
        with tc.tile_pool(name="const", bufs=1) as cp:
            # ---- persistent SBUF tensors ----
            xT = cp.tile([128, 6, T], bf16, tag="xT")
            wqk = cp.tile([128, 6, 2 * C], bf16, tag="wqk")
            wv = cp.tile([128, 6, C], bf16, tag="wv")
            wpr = cp.tile([128, 6, C], bf16, tag="wpr")
            bqk = cp.tile([128, 12], f32, tag="bqk")
            bv_row = cp.tile([1, C], f32, tag="bv_row")
            bp_row = cp.tile([1, C], f32, tag="bp_row")
            bv_bc = cp.tile([128, C], f32, tag="bv_bc")
            bp_bc = cp.tile([128, C], f32, tag="bp_bc")
            relt = cp.tile([64, 2048], bf16, tag="relt")
            qext = cp.tile([128, NH, 32, 32], bf16, tag="qext")
            kext = cp.tile([128, NH, T], bf16, tag="kext")
            vaug = cp.tile([128, 8, NH, 65], bf16, tag="vaug")
            yall = cp.tile([128, 6, T], bf16, tag="yall")

            # ---- input DMAs ----
            for c in range(6):
                nc.sync.dma_start(xT[:, c, :], xT_d[c * 128:(c + 1) * 128, :])
            for c in range(6):
                nc.sync.dma_start(wqk[:, c, :], wqk_d[c * 128:(c + 1) * 128, :])
            for c in range(6):
                nc.sync.dma_start(wv[:, c, :], wv_d[c * 128:(c + 1) * 128, :])
            for c in range(6):
                nc.sync.dma_start(wpr[:, c, :], wp_d[c * 128:(c + 1) * 128, :])
            nc.sync.dma_start(bqk[:], bqk_d[:])
            nc.sync.dma_start(bv_row[:], bv_d[:])
            nc.sync.dma_start(bp_row[:], bp_d[:])
            nc.sync.dma_start(relt[:], relt_d[:])
            nc.gpsimd.partition_broadcast(bv_bc[:], bv_row[:])
            nc.gpsimd.partition_broadcast(bp_bc[:], bp_row[:])
            # onehot block straight into each head's Kext rows 64:128
            for h in range(NH):
                nc.sync.dma_start(kext[64:128, h, :], oneh_d[:])
            # ones column of Vaug (softmax denominator)
            nc.gpsimd.memset(vaug[:, :, :, 64:65], 1.0)

            # ======== phase 1a: q projections (o-tiles 0-5) ========
            def qk_otile(ps, ot):
                acc = ps.tile([128, T], f32, tag="qk")
                for c in range(6):
                    for hf in range(2):
                        nc.tensor.matmul(
                            acc[:, hf * 512:(hf + 1) * 512],
                            wqk[:, c, ot * 128:(ot + 1) * 128],
                            xT[:, c, hf * 512:(hf + 1) * 512],
                            start=(c == 0), stop=(c == 5),
                        )
                is_q = ot < 6
                hp = ot if is_q else ot - 6  # head pair index
                for half in range(2):
                    head = 2 * hp + half
                    src = acc[64 * half:64 * (half + 1), :]
                    bias = bqk[64 * half:64 * (half + 1), ot:ot + 1]
                    if is_q:
                        dst = qext[0:64, head, :, :]
                    else:
                        dst = kext[0:64, head, :]
                    nc.vector.tensor_scalar_add(dst, src, bias)

            with tc.tile_pool(name="ps_qk", bufs=2, space="PSUM") as ps_qk:
                for ot in range(6):
                    qk_otile(ps_qk, ot)

                # ==== phase 1b: rel-pos tables, overlapped with k o-tiles ====
                # Bands m=2 (rel_h -> qext rows 64:96) and m=3 (rel_w ->
                # rows 96:128) so every evacuation is partition-aligned and
                # can run on either ScalarE or VectorE. k o-tiles are
                # interleaved so the PE stays warm while DVE/ACT drain the
                # rel psum tiles.
                k_sched = {1: 6, 4: 7, 7: 8, 10: 9, 12: 10, 14: 11}
                with tc.tile_pool(name="ps_rel", bufs=2, space="PSUM") as ps_rel:
                    for i in range(16):
                        if i in k_sched:
                            qk_otile(ps_qk, k_sched[i])
                        accr = ps_rel.tile([128, 2, 512], f32, tag="rel")
                        for g in range(2):
                            qx = 2 * i + g
                            for tbl in range(2):
                                m = 2 + tbl
                                lhsT = relt[0:64, tbl * 1024 + qx * 32:
                                            tbl * 1024 + qx * 32 + 32]
                                rhs = (qext[0:64, :, qx, :] if tbl == 0
                                       else qext[0:64, :, :, qx])
                                nc.tensor.matmul(
                                    accr[32 * m:32 * (m + 1), g, 0:NH * 32],
                                    lhsT, rhs,
                                    start=True, stop=True,
                                    tile_position=(0, 32 * m),
                                )
                        q0 = 2 * i
                        src_h = accr[64:96, :, 0:NH * 32]
                        dst_h = qext[64:96, :, q0:q0 + 2, :].rearrange(
                            "p h a b -> p a h b")
                        src_w = accr[96:128, :, 0:NH * 32]
                        dst_w = qext[96:128, :, :, q0:q0 + 2].rearrange(
                            "p h a b -> p b h a")
                        if i % 2 == 0:
                            nc.scalar.copy(dst_h, src_h)
                            nc.vector.tensor_copy(dst_w, src_w)
                        else:
                            nc.vector.tensor_copy(dst_h, src_h)
                            nc.scalar.copy(dst_w, src_w)

            # ======== phase 1c: v GEMM ========
            with tc.tile_pool(name="ps_v", bufs=2, space="PSUM") as ps_v:
                for tt in range(8):
                    accv = ps_v.tile([128, C], f32, tag="v")
                    for c in range(6):
                        nc.tensor.matmul(
                            accv[:, 0:512],
                            xT[:, c, tt * 128:(tt + 1) * 128],
                            wv[:, c, 0:512],
                            start=(c == 0), stop=(c == 5),
                        )
                        nc.tensor.matmul(
                            accv[:, 512:768],
                            xT[:, c, tt * 128:(tt + 1) * 128],
                            wv[:, c, 512:768],
                            start=(c == 0), stop=(c == 5),
                        )
                    nc.vector.tensor_add(
                        vaug[:, tt, :, 0:64], accv[:], bv_bc[:])

            # ================= phase 2b: attention per head =================
            with tc.tile_pool(name="ps_s", bufs=2, space="PSUM") as ps_s, \
                 tc.tile_pool(name="ps_pv", bufs=2, space="PSUM") as ps_pv, \
                 tc.tile_pool(name="attn_sb", bufs=2) as asb:
                for h in range(NH):
                    p_t = asb.tile([128, 8, T], bf16, tag="P")
                    for kt in range(8):
                        accs = ps_s.tile([128, T], f32, tag="S")
                        for hf in range(2):
                            nc.tensor.matmul(
                                accs[:, hf * 512:(hf + 1) * 512],
                                kext[:, h, kt * 128:(kt + 1) * 128],
                                qext[:, h, hf * 16:(hf + 1) * 16, :],
                                start=True, stop=True,
                            )
                        nc.scalar.activation(p_t[:, kt, :], accs[:], EXP)
                    accp = ps_pv.tile([65, T], f32, tag="PV")
                    for kt in range(8):
                        for hf in range(2):
                            nc.tensor.matmul(
                                accp[:, hf * 512:(hf + 1) * 512],
                                vaug[:, kt, h, :],
                                p_t[:, kt, hf * 512:(hf + 1) * 512],
                                start=(kt == 0), stop=(kt == 7),
                            )
                    # normalization: reciprocal costs ~6.4 DVE cycles/elem,
                    # so spread the 1024 denominators over 32 lanes via a
                    # small DMA reshape (32 descriptors each way) first.
                    d_row = asb.tile([1, T], f32, tag="d")
                    d_sq = asb.tile([32, 32], f32, tag="dsq")
                    r_row = asb.tile([1, T], f32, tag="r")
                    r_bc = asb.tile([64, T], f32, tag="rbc")
                    nc.vector.tensor_copy(d_row[:], accp[64:65, :])
                    nc.sync.dma_start(d_sq[:], d_row[:])
                    nc.vector.reciprocal(d_sq[:], d_sq[:])
                    nc.sync.dma_start(r_row[:], d_sq[:])
                    nc.gpsimd.partition_broadcast(r_bc[:], r_row[:])
                    nc.vector.tensor_mul(
                        yall[64 * (h % 2):64 * (h % 2 + 1), h // 2, :],
                        accp[0:64, :], r_bc[:])

            # ================= phase 3: output projection =================
            with tc.tile_pool(name="ps_pr", bufs=2, space="PSUM") as ps_pr, \
                 tc.tile_pool(name="proj_sb", bufs=2) as psb:
                for tt in range(8):
                    accz = ps_pr.tile([128, C], f32, tag="Z")
                    for p in range(6):
                        nc.tensor.matmul(
                            accz[:, 0:512],
                            yall[:, p, tt * 128:(tt + 1) * 128],
                            wpr[:, p, 0:512],
                            start=(p == 0), stop=(p == 5),
                        )
                        nc.tensor.matmul(
                            accz[:, 512:768],
                            yall[:, p, tt * 128:(tt + 1) * 128],
                            wpr[:, p, 512:768],
                            start=(p == 0), stop=(p == 5),
                        )
                    z_t = psb.tile([128, C], f32, tag="Zt")
                    nc.vector.tensor_add(z_t[:], accz[:], bp_bc[:])
                    nc.sync.dma_start(out_d[tt * 128:(tt + 1) * 128, :], z_t[:])

    nc.compile()
    _cache["nc"] = nc
    return nc


def _host_prep(x, w_qkv, b_qkv, w_proj, b_proj, rel_pos_h, rel_pos_w):
    scale = HD ** -0.5
    w_qkv = _f32(w_qkv)
    b_qkv = _f32(b_qkv)

    w_qk = w_qkv[:, : 2 * C].copy()
    w_qk[:, :C] *= scale
    b_qk_flat = b_qkv[: 2 * C].copy()
    b_qk_flat[:C] *= scale
    b_qk = np.ascontiguousarray(b_qk_flat.reshape(12, 128).T)  # [128, 12]

    # relt [64, 2048]: cols tbl*1024 + qx*32 + j -> 8*rel_pos[qx - j + 31, :]
    idx = np.arange(32)[:, None] - np.arange(32)[None, :] + 31  # [qx, j]
    relt = np.concatenate(
        [
            (8.0 * _f32(rel_pos_h))[idx].transpose(2, 0, 1).reshape(64, 1024),
            (8.0 * _f32(rel_pos_w))[idx].transpose(2, 0, 1).reshape(64, 1024),
        ],
        axis=1,
    )

    k = np.arange(T)
    onehot = np.zeros((64, T), np.float32)
    onehot[k // 32, k] = 1.0
    onehot[32 + k % 32, k] = 1.0

    shared = {
        "w_qk": _bf(w_qk),
        "w_v": _bf(w_qkv[:, 2 * C:]),
        "w_p": _bf(w_proj),
        "b_qk": _f32(b_qk),
        "b_v": _f32(b_qkv[2 * C:])[None, :],
        "b_p": _f32(b_proj)[None, :],
        "relt": _bf(relt),
        "onehot": _bf(onehot),
    }
    x = _f32(x)
    in_maps = []
    for i in range(N_CORES):
        m = dict(shared)
        m["xT"] = _bf(x[i].reshape(T, C).T)
        in_maps.append(m)
    return in_maps


def kernel(x, w_qkv, b_qkv, w_proj, b_proj, rel_pos_h, rel_pos_w):
    from concourse.bass_utils import run_bass_kernel_spmd

    nc = _build_nc()
    in_maps = _host_prep(x, w_qkv, b_qkv, w_proj, b_proj, rel_pos_h, rel_pos_w)
    res = run_bass_kernel_spmd(nc, in_maps, core_ids=list(range(N_CORES)))
    out = np.stack([_f32(res.results[i]["out"]) for i in range(N_CORES)])
    return out.reshape(B, H, W, C)


# revision 22
# speedup vs baseline: 1.2142x; 1.0020x over previous
"""ViT-style attention with decomposed relative position embeddings on 8 TRN2
NeuronCores. Data-parallel over batch (B=8 -> 1 image per core); weights and
the small rel-pos tables are replicated.

Per-core computation (one image, T=1024 tokens, C=768, 12 heads x 64):
  - qk^T GEMM:  qk^T[o, t] = w_qk^T . x^T   (q pre-scaled by 1/8 host-side)
  - v GEMM:     V[t, o] = x . w_v + b_v
  - rel-pos fold: logits = [q; rel_h_pre; rel_w_pre] . [k; onehot_h; onehot_w]
    so the decomposed rel-pos additions ride in the same 128-deep matmul
    contraction as q.k (the head-dim is only 64, so the extra 64 contraction
    rows are free on the 128x128 PE array).
  - S^T = Kext^T . Qext per head, exp on ScalarE -> P^T (bf16)
  - out^T[n, q] = Vaug^T . P^T accumulated over k-chunks; the appended ones
    column of Vaug makes row 64 the softmax denominator for free.
  - normalization: the denominator row is DMA-reshaped [1,1024]->[32,32] so
    the expensive DVE reciprocal (~6.4 cyc/elem) runs on 32 lanes instead of
    one, then gpsimd partition-broadcast + DVE multiply.
  - proj GEMM + bias -> out

All matmuls run in bf16 (fp32 PSUM accumulation).
"""

import numpy as np
import ml_dtypes

BF16 = ml_dtypes.bfloat16

B, H, W, C = 8, 32, 32, 768
NH, HD, T = 12, 64, 1024
N_CORES = 8

_cache = {}


def _bf(a):
    return np.ascontiguousarray(np.asarray(a, dtype=np.float32)).astype(BF16)


def _f32(a):
    return np.ascontiguousarray(np.asarray(a, dtype=np.float32))


def _build_nc():
    if "nc" in _cache:
        return _cache["nc"]

    import concourse.mybir as mybir
    import concourse.tile as tile
    from concourse import bacc

    f32 = mybir.dt.float32
    bf16 = mybir.dt.bfloat16
    EXP = mybir.ActivationFunctionType.Exp

    nc = bacc.Bacc("TRN2", target_bir_lowering=False, debug=False)

    # ---- DRAM I/O ----
    xT_d = nc.dram_tensor("xT", [C, T], bf16, kind="ExternalInput")
    wqk_d = nc.dram_tensor("w_qk", [C, 2 * C], bf16, kind="ExternalInput")
    wv_d = nc.dram_tensor("w_v", [C, C], bf16, kind="ExternalInput")
    wp_d = nc.dram_tensor("w_p", [C, C], bf16, kind="ExternalInput")
    bqk_d = nc.dram_tensor("b_qk", [128, 12], f32, kind="ExternalInput")
    bv_d = nc.dram_tensor("b_v", [1, C], f32, kind="ExternalInput")
    bp_d = nc.dram_tensor("b_p", [1, C], f32, kind="ExternalInput")
    relt_d = nc.dram_tensor("relt", [64, 2048], bf16, kind="ExternalInput")
    oneh_d = nc.dram_tensor("onehot", [64, T], bf16, kind="ExternalInput")
    out_d = nc.dram_tensor("out", [T, C], f32, kind="ExternalOutput")

    with tile.TileContext(nc) as tc:
        with tc.tile_pool(name="const", bufs=1) as cp:
            # ---- persistent SBUF tensors ----
            xT = cp.tile([128, 6, T], bf16, tag="xT")
            wqk = cp.tile([128, 6, 2 * C], bf16, tag="wqk")
            wv = cp.tile([128, 6, C], bf16, tag="wv")
            wpr = cp.tile([128, 6, C], bf16, tag="wpr")
            bqk = cp.tile([128, 12], f32, tag="bqk")
            bv_row = cp.tile([1, C], f32, tag="bv_row")
            bp_row = cp.tile([1, C], f32, tag="bp_row")
            bv_bc = cp.tile([128, C], f32, tag="bv_bc")
            bp_bc = cp.tile([128, C], f32, tag="bp_bc")
            relt = cp.tile([64, 2048], bf16, tag="relt")
            qext = cp.tile([128, NH, 32, 32], bf16, tag="qext")
            kext = cp.tile([128, NH, T], bf16, tag="kext")
            vaug = cp.tile([128, 8, NH, 65], bf16, tag="vaug")
            yall = cp.tile([128, 6, T], bf16, tag="yall")

            # ---- input DMAs (interleaved so matmul (ot=0, c=0) can
            # start as soon as the first xT and wqk chunks land) ----
            for c in range(6):
                nc.sync.dma_start(xT[:, c, :], xT_d[c * 128:(c + 1) * 128, :])
                nc.sync.dma_start(wqk[:, c, :], wqk_d[c * 128:(c + 1) * 128, :])
            for c in range(6):
                nc.sync.dma_start(wv[:, c, :], wv_d[c * 128:(c + 1) * 128, :])
            for c in range(6):
                nc.sync.dma_start(wpr[:, c, :], wp_d[c * 128:(c + 1) * 128, :])
            nc.sync.dma_start(bqk[:], bqk_d[:])
            nc.sync.dma_start(bv_row[:], bv_d[:])
            nc.sync.dma_start(bp_row[:], bp_d[:])
            nc.sync.dma_start(relt[:], relt_d[:])
            nc.gpsimd.partition_broadcast(bv_bc[:], bv_row[:])
            nc.gpsimd.partition_broadcast(bp_bc[:], bp_row[:])
            # onehot block straight into each head's Kext rows 64:128
            for h in range(NH):
                nc.sync.dma_start(kext[64:128, h, :], oneh_d[:])
            # ones column of Vaug (softmax denominator)
            nc.gpsimd.memset(vaug[:, :, :, 64:65], 1.0)

            # ======== phase 1a: q projections (o-tiles 0-5) ========
            def qk_otile(ps, ot):
                acc = ps.tile([128, T], f32, tag="qk")
                for c in range(6):
                    for hf in range(2):
                        nc.tensor.matmul(
                            acc[:, hf * 512:(hf + 1) * 512],
                            wqk[:, c, ot * 128:(ot + 1) * 128],
                            xT[:, c, hf * 512:(hf + 1) * 512],
                            start=(c == 0), stop=(c == 5),
                        )
                is_q = ot < 6
                hp = ot if is_q else ot - 6  # head pair index
                for half in range(2):
                    head = 2 * hp + half
                    src = acc[64 * half:64 * (half + 1), :]
                    bias = bqk[64 * half:64 * (half + 1), ot:ot + 1]
                    if is_q:
                        dst = qext[0:64, head, :, :]
                    else:
                        dst = kext[0:64, head, :]
                    nc.vector.tensor_scalar_add(dst, src, bias)

            with tc.tile_pool(name="ps_qk", bufs=2, space="PSUM") as ps_qk:
                for ot in range(6):
                    qk_otile(ps_qk, ot)

                # ==== phase 1b: rel-pos tables, overlapped with k o-tiles ====
                # Bands m=2 (rel_h -> qext rows 64:96) and m=3 (rel_w ->
                # rows 96:128) so every evacuation is partition-aligned and
                # can run on either ScalarE or VectorE. k o-tiles are
                # interleaved so the PE stays warm while DVE/ACT drain the
                # rel psum tiles.
                k_sched = {1: 6, 4: 7, 7: 8, 10: 9, 12: 10, 14: 11}
                with tc.tile_pool(name="ps_rel", bufs=2, space="PSUM") as ps_rel:
                    for i in range(16):
                        if i in k_sched:
                            qk_otile(ps_qk, k_sched[i])
                        accr = ps_rel.tile([128, 2, 512], f32, tag="rel")
                        for g in range(2):
                            qx = 2 * i + g
                            for tbl in range(2):
                                m = 2 + tbl
                                lhsT = relt[0:64, tbl * 1024 + qx * 32:
                                            tbl * 1024 + qx * 32 + 32]
                                rhs = (qext[0:64, :, qx, :] if tbl == 0
                                       else qext[0:64, :, :, qx])
                                nc.tensor.matmul(
                                    accr[32 * m:32 * (m + 1), g, 0:NH * 32],
                                    lhsT, rhs,
                                    start=True, stop=True,
                                    tile_position=(0, 32 * m),
                                )
                        q0 = 2 * i
                        src_h = accr[64:96, :, 0:NH * 32]
                        dst_h = qext[64:96, :, q0:q0 + 2, :].rearrange(
                            "p h a b -> p a h b")
                        src_w = accr[96:128, :, 0:NH * 32]
                        dst_w = qext[96:128, :, :, q0:q0 + 2].rearrange(
                            "p h a b -> p b h a")
                        if i % 2 == 0:
                            nc.scalar.copy(dst_h, src_h)
                            nc.vector.tensor_copy(dst_w, src_w)
                        else:
                            nc.vector.tensor_copy(dst_h, src_h)
                            nc.scalar.copy(dst_w, src_w)

            # ======== phase 1c: v GEMM ========
            with tc.tile_pool(name="ps_v", bufs=2, space="PSUM") as ps_v:
                for tt in range(8):
                    accv = ps_v.tile([128, C], f32, tag="v")
                    for c in range(6):
                        nc.tensor.matmul(
                            accv[:, 0:512],
                            xT[:, c, tt * 128:(tt + 1) * 128],
                            wv[:, c, 0:512],
                            start=(c == 0), stop=(c == 5),
                        )
                        nc.tensor.matmul(
                            accv[:, 512:768],
                            xT[:, c, tt * 128:(tt + 1) * 128],
                            wv[:, c, 512:768],
                            start=(c == 0), stop=(c == 5),
                        )
                    nc.vector.tensor_add(
                        vaug[:, tt, :, 0:64], accv[:], bv_bc[:])

            # ================= phase 2b: attention per head =================
            with tc.tile_pool(name="ps_s", bufs=2, space="PSUM") as ps_s, \
                 tc.tile_pool(name="ps_pv", bufs=2, space="PSUM") as ps_pv, \
                 tc.tile_pool(name="attn_sb", bufs=2) as asb:
                for h in range(NH):
                    p_t = asb.tile([128, 8, T], bf16, tag="P")
                    for kt in range(8):
                        accs = ps_s.tile([128, T], f32, tag="S")
                        for hf in range(2):
                            nc.tensor.matmul(
                                accs[:, hf * 512:(hf + 1) * 512],
                                kext[:, h, kt * 128:(kt + 1) * 128],
                                qext[:, h, hf * 16:(hf + 1) * 16, :],
                                start=True, stop=True,
                            )
                        nc.scalar.activation(p_t[:, kt, :], accs[:], EXP)
                    accp = ps_pv.tile([65, T], f32, tag="PV")
                    for kt in range(8):
                        for hf in range(2):
                            nc.tensor.matmul(
                                accp[:, hf * 512:(hf + 1) * 512],
                                vaug[:, kt, h, :],
                                p_t[:, kt, hf * 512:(hf + 1) * 512],
                                start=(kt == 0), stop=(kt == 7),
                            )
                    # normalization: reciprocal costs ~6.4 DVE cycles/elem,
                    # so spread the 1024 denominators over 32 lanes via a
                    # small DMA reshape (32 descriptors each way) first.
                    d_row = asb.tile([1, T], f32, tag="d")
                    d_sq = asb.tile([32, 32], f32, tag="dsq")
                    r_row = asb.tile([1, T], f32, tag="r")
                    r_bc = asb.tile([64, T], f32, tag="rbc")
                    nc.vector.tensor_copy(d_row[:], accp[64:65, :])
                    nc.sync.dma_start(d_sq[:], d_row[:])
                    nc.vector.reciprocal(d_sq[:], d_sq[:])
                    nc.sync.dma_start(r_row[:], d_sq[:])
                    nc.gpsimd.partition_broadcast(r_bc[:], r_row[:])
                    nc.vector.tensor_mul(
                        yall[64 * (h % 2):64 * (h % 2 + 1), h // 2, :],
                        accp[0:64, :], r_bc[:])

            # ================= phase 3: output projection =================
            with tc.tile_pool(name="ps_pr", bufs=2, space="PSUM") as ps_pr, \
                 tc.tile_pool(name="proj_sb", bufs=2) as psb:
                for tt in range(8):
                    accz = ps_pr.tile([128, C], f32, tag="Z")
                    for p in range(6):
                        nc.tensor.matmul(
                            accz[:, 0:512],
                            yall[:, p, tt * 128:(tt + 1) * 128],
                            wpr[:, p, 0:512],
                            start=(p == 0), stop=(p == 5),
                        )
                        nc.tensor.matmul(
                            accz[:, 512:768],
                            yall[:, p, tt * 128:(tt + 1) * 128],
                            wpr[:, p, 512:768],
                            start=(p == 0), stop=(p == 5),
                        )
                    z_t = psb.tile([128, C], f32, tag="Zt")
                    nc.vector.tensor_add(z_t[:], accz[:], bp_bc[:])
                    nc.sync.dma_start(out_d[tt * 128:(tt + 1) * 128, :], z_t[:])

    nc.compile()
    _cache["nc"] = nc
    return nc


def _host_prep(x, w_qkv, b_qkv, w_proj, b_proj, rel_pos_h, rel_pos_w):
    scale = HD ** -0.5
    w_qkv = _f32(w_qkv)
    b_qkv = _f32(b_qkv)

    w_qk = w_qkv[:, : 2 * C].copy()
    w_qk[:, :C] *= scale
    b_qk_flat = b_qkv[: 2 * C].copy()
    b_qk_flat[:C] *= scale
    b_qk = np.ascontiguousarray(b_qk_flat.reshape(12, 128).T)  # [128, 12]

    # relt [64, 2048]: cols tbl*1024 + qx*32 + j -> 8*rel_pos[qx - j + 31, :]
    idx = np.arange(32)[:, None] - np.arange(32)[None, :] + 31  # [qx, j]
    relt = np.concatenate(
        [
            (8.0 * _f32(rel_pos_h))[idx].transpose(2, 0, 1).reshape(64, 1024),
            (8.0 * _f32(rel_pos_w))[idx].transpose(2, 0, 1).reshape(64, 1024),
        ],
        axis=1,
    )

    k = np.arange(T)
    onehot = np.zeros((64, T), np.float32)
    onehot[k // 32, k] = 1.0
    onehot[32 + k % 32, k] = 1.0

    shared = {
        "w_qk": _bf(w_qk),
        "w_v": _bf(w_qkv[:, 2 * C:]),
        "w_p": _bf(w_proj),
        "b_qk": _f32(b_qk),
        "b_v": _f32(b_qkv[2 * C:])[None, :],
        "b_p": _f32(b_proj)[None, :],
        "relt": _bf(relt),
        "onehot": _bf(onehot),
    }
    x = _f32(x)
    in_maps = []
    for i in range(N_CORES):
        m = dict(shared)
        m["xT"] = _bf(x[i].reshape(T, C).T)
        in_maps.append(m)
    return in_maps


def kernel(x, w_qkv, b_qkv, w_proj, b_proj, rel_pos_h, rel_pos_w):
    from concourse.bass_utils import run_bass_kernel_spmd

    nc = _build_nc()
    in_maps = _host_prep(x, w_qkv, b_qkv, w_proj, b_proj, rel_pos_h, rel_pos_w)
    res = run_bass_kernel_spmd(nc, in_maps, core_ids=list(range(N_CORES)))
    out = np.stack([_f32(res.results[i]["out"]) for i in range(N_CORES)])
    return out.reshape(B, H, W, C)
